# revision 1
# baseline (speedup 1.0000x reference)
"""DSNT double-loss kernel for Trainium2 (8 NeuronCores, data-parallel over B).

Per core: 64 heatmaps (4 batches x 16 ch), each 256x256 = 65536 px.
On-chip heatmap layout [128 part, 512 free]: flat pixel = 512*p + c,
h = 2p + (c>=256), w = c % 256.

DRAM layout per core (host-packed):
  input  [128, 64*512] fp8e4 (col = hm*512 + c)   ~4.2 MB
  target [128, 64*512] f32                        ~16.8 MB
  consts [128, 336]    f32 (single packed DMA)

Pipeline (all stream DMAs issued on the Sync queue, FIFO):
  interleaved input/target chunks sized so ACT exp (~29.5us) spreads over
  the ~61us stream and the tail stays short; stats accumulate in ONE PSUM
  bank (no per-heatmap copies); stage-3 column folds run incrementally per
  input chunk; target argmax resolution runs per 32-heatmap half (the
  first half's masked-min + 32-row gather + max_index hides mid-stream).
  Device returns ed^2 [64]; host does sqrt + 8-way sum + /B.
"""

import numpy as np
from contextlib import ExitStack

import concourse.bass as bass
import concourse.bacc as bacc
import concourse.tile as tile
from concourse import mybir
from concourse.bass_utils import run_bass_kernel_spmd

F32 = mybir.dt.float32
BF16 = mybir.dt.bfloat16
F8 = mybir.dt.float8e4
U16 = mybir.dt.uint16
I16 = mybir.dt.int16
OP = mybir.AluOpType
AX = mybir.AxisListType
AF = mybir.ActivationFunctionType

B, CH, H, W = 32, 16, 256, 256
NCORES = 8
BPC = B // NCORES          # 4 batches per core
NHM = BPC * CH             # 64 heatmaps per core
P, C = 128, 512            # on-chip heatmap tile shape
TOTC = NHM * C             # 32768 cols

# stream schedule, in heatmaps: (kind, h0, nh)
ICHUNKS = [16, 16, 8, 8, 8, 4, 4]
TCHUNKS = [8, 8, 8, 8, 8, 8, 6, 6, 4]
ORDER = ["I0", "T0", "I1", "T1", "T2", "I2", "T3", "T4", "I3",
         "T5", "I4", "T6", "T7", "T8", "I5", "I6"]

NCC = 416  # const cols


def make_consts():
    p = np.arange(128, dtype=np.float32)
    cw = np.zeros((128, NCC), dtype=np.float32)
    cw[:, 0:128] = np.eye(128, dtype=np.float32)          # ident
    cw[:, 128] = 1.0                                      # r3A ones
    cw[:, 129] = (2.0 * p - 255.0) / 256.0                # r3A xsA
    cw[:, 130] = 1.0                                      # r3B ones
    cw[:, 131] = (2.0 * p + 1.0) / 256.0                  # r3B xsB
    cw[:, 132] = 1.0                                      # onesc
    cw[:, 133] = 1.0                                      # wE2 ones
    cw[:, 134] = (4.0 * p - 255.0) / 256.0                # wE2 y-even
    cw[:, 135] = 1.0                                      # wO2 ones
    cw[:, 136] = (4.0 * p - 253.0) / 256.0                # wO2 y-odd
    # [64, *] consts in partitions 0-63
    cw[0:64, 137:265] = p[None, 0:128] + 65536.0          # cpb
    cw[0:32, 265] = np.arange(32, dtype=np.float32)       # hmidx half 0
    cw[0:32, 404] = np.arange(32, dtype=np.float32) + 32  # hmidx half 1
    cw[0:64, 266:274] = 1.0                               # ones [64,8]
    i32 = np.arange(32)
    cw[0:32, 274:276] = (i32[:, None] // 16 == np.arange(2)[None, :])  # Mwrap32
    # PERM: idx i -> partition i%16, replicated across the 8 gpsimd cores
    cw[0:32, 276:404] = (i32[:, None] % 16 == np.arange(128)[None, :] % 16)
    return cw


def build_nc(debug=False):
    nc = bacc.Bacc(
        "TRN2",
        target_bir_lowering=False,
        debug=False,
        enable_asserts=False,
        num_devices=NCORES,
    )
    inp = nc.dram_tensor("input", [P, TOTC], F8, kind="ExternalInput").ap()
    tgt = nc.dram_tensor("target", [P, TOTC], F32, kind="ExternalInput").ap()
    cdram = nc.dram_tensor("consts", [P, NCC], F32, kind="ExternalInput").ap()
    out = nc.dram_tensor("out", [NHM, 1], F32, kind="ExternalOutput").ap()
    tgt_rows = tgt.rearrange("p (h c) -> (p h) c", c=C)   # row r = p*64 + hm

    with ExitStack() as ctx:
        tc = ctx.enter_context(tile.TileContext(nc))
        cpool = ctx.enter_context(tc.tile_pool(name="consts", bufs=1))
        inpool = ctx.enter_context(tc.tile_pool(name="inp", bufs=2))
        tpool = ctx.enter_context(tc.tile_pool(name="tgt", bufs=4))
        epool = ctx.enter_context(tc.tile_pool(name="e", bufs=2))
        spool = ctx.enter_context(tc.tile_pool(name="stats", bufs=1))
        fpool = ctx.enter_context(tc.tile_pool(name="fin", bufs=1))
        warmp = ctx.enter_context(tc.tile_pool(name="warm", bufs=1))
        statsps = ctx.enter_context(tc.tile_pool(name="statsps", bufs=1, space="PSUM"))
        s12ps = ctx.enter_context(tc.tile_pool(name="s12ps", bufs=1, space="PSUM"))
        mmps = ctx.enter_context(tc.tile_pool(name="mmps", bufs=1, space="PSUM"))

        # ---- interleave schedule bookkeeping
        ioff = np.cumsum([0] + ICHUNKS)
        toff = np.cumsum([0] + TCHUNKS)

        # issue first data DMAs before consts so the stream starts instantly
        it = {}
        tt = {}

        def issue_dma(nm):
            k = int(nm[1:])
            if nm[0] == "I":
                h0, nh = int(ioff[k]), ICHUNKS[k]
                t = inpool.tile([P, max(ICHUNKS) * C], F8, tag="it")
                nc.sync.dma_start(t[:, 0:nh * C], inp[:, h0 * C:(h0 + nh) * C])
                it[k] = (t, h0, nh)
            else:
                h0, nh = int(toff[k]), TCHUNKS[k]
                t = tpool.tile([P, max(TCHUNKS) * C], F32, tag="tt")
                nc.sync.dma_start(t[:, 0:nh * C], tgt[:, h0 * C:(h0 + nh) * C])
                tt[k] = (t, h0, nh)

        issue_dma(ORDER[0])          # I0
        issue_dma(ORDER[1])          # T0
        cw = cpool.tile([P, NCC], F32, tag="cw")
        nc.sync.dma_start(cw[:], cdram)
        for nm in ORDER[2:]:
            issue_dma(nm)

        # bf16 stage-1 weights from the f32 const block
        wE2 = cpool.tile([128, 2], BF16, tag="wE2")
        nc.vector.tensor_copy(wE2[:], cw[:, 133:135])
        wO2 = cpool.tile([128, 2], BF16, tag="wO2")
        nc.vector.tensor_copy(wO2[:], cw[:, 135:137])

        stats = spool.tile([128, 4 * NHM], F32, tag="stats")      # SBUF copy
        statsp = statsps.tile([128, 4 * NHM], F32, tag="statsp")  # one PSUM bank
        S12 = [s12ps.tile([32, 3], F32, tag=f"S12_{g}", name=f"S12_{g}")
               for g in range(2)]
        RM = spool.tile([128, NHM], F32, tag="RM")

        # ---- warm the gpsimd DGE gather library early (overlaps stream)
        zidx = warmp.tile([128, 2], I16, tag="zidx")
        nc.gpsimd.memset(zidx[:], 0)
        gwarm = warmp.tile([128, C], F32, tag="gwarm")
        nc.gpsimd.dma_gather(
            gwarm[:].rearrange("p (o c) -> p o c", o=1),
            tgt_rows, zidx[:], num_idxs=32, num_idxs_reg=32, elem_size=C,
        )

        # ---- per-chunk compute emission (Tile resolves the actual overlap)
        def input_compute(k):
            t, h0, nh = it[k]
            et = epool.tile([P, max(ICHUNKS) * C], BF16, tag="et")
            # exp in <=4-heatmap slices to keep ACT granular
            for s0 in range(0, nh, 4):
                sn = min(4, nh - s0)
                nc.scalar.activation(et[:, s0 * C:(s0 + sn) * C],
                                     t[:, s0 * C:(s0 + sn) * C], AF.Exp)
            for j in range(nh):
                hm = h0 + j
                base = j * C
                pscol = 4 * hm
                nc.tensor.matmul(statsp[:, pscol:pscol + 2],
                                 et[:, base + 0:base + 128], wE2[:],
                                 start=True, stop=False)
                nc.tensor.matmul(statsp[:, pscol:pscol + 2],
                                 et[:, base + 256:base + 384], wO2[:],
                                 start=False, stop=True)
                nc.tensor.matmul(statsp[:, pscol + 2:pscol + 4],
                                 et[:, base + 128:base + 256], wE2[:],
                                 start=True, stop=False)
                nc.tensor.matmul(statsp[:, pscol + 2:pscol + 4],
                                 et[:, base + 384:base + 512], wO2[:],
                                 start=False, stop=True)
            # incremental PSUM -> SBUF stats copy for this hm range
            c0, c1 = 4 * h0, 4 * (h0 + nh)
            nc.vector.tensor_copy(stats[:, c0:c1], statsp[:, c0:c1])
            # stage 3 per completed 32-heatmap half
            for g in range(2):
                if h0 + nh == 32 * (g + 1):
                    d0, d1 = 128 * g, 128 * (g + 1)
                    a0 = stats[:, d0 + 0:d1:4]
                    a1 = stats[:, d0 + 1:d1:4]
                    b0 = stats[:, d0 + 2:d1:4]
                    b1 = stats[:, d0 + 3:d1:4]
                    nc.tensor.matmul(S12[g][:, 0:2], a0, cw[:, 128:130],
                                     start=True, stop=False)
                    nc.tensor.matmul(S12[g][:, 0:2], b0, cw[:, 130:132],
                                     start=False, stop=True)
                    nc.tensor.matmul(S12[g][:, 2:3], a1, cw[:, 132:133],
                                     start=True, stop=False)
                    nc.tensor.matmul(S12[g][:, 2:3], b1, cw[:, 132:133],
                                     start=False, stop=True)

        def target_compute(k):
            t, h0, nh = tt[k]
            # row maxima in <=2-heatmap sub-reduces
            for i, s0 in enumerate(range(0, nh, 2)):
                sn = min(2, nh - s0)
                eng = nc.vector
                eng.tensor_reduce(
                    RM[:, h0 + s0:h0 + s0 + sn],
                    t[:, s0 * C:(s0 + sn) * C].rearrange(
                        "p (n c) -> p n c", n=sn),
                    axis=AX.X, op=OP.max,
                )

        # ---- per-half argmax resolution + ed^2, emitted in two pieces so
        # in-order engines never stall the stream (A: mask/pstar/index +
        # gather launch; B: cstar/coords/ed2 once the gather surely landed)
        ed2 = [fpool.tile([32, 1], F32, tag=f"ed2_{g}", name=f"ed2_{g}")
               for g in range(2)]
        half = [{} for _ in range(2)]

        def res_a(g):
            g0 = 32 * g
            st = half[g]
            RMT = mmps.tile([32, 128], F32, tag=f"RMT{g}", name=f"RMT{g}")
            nc.tensor.transpose(RMT[:], RM[:, g0:g0 + 32], cw[:, 0:128])
            RMTs = fpool.tile([32, 128], F32, tag=f"RMTs{g}", name=f"RMTs{g}")
            nc.vector.tensor_copy(RMTs[:], RMT[:])
            mh = fpool.tile([32, 1], F32, tag=f"mh{g}", name=f"mh{g}")
            nc.vector.reduce_max(mh[:], RMTs[:], axis=AX.X)
            mp = fpool.tile([32, 128], F32, tag=f"mp{g}", name=f"mp{g}")
            nc.vector.tensor_scalar(mp[:], RMTs[:], mh[:], None, op0=OP.is_ge)
            selp = fpool.tile([32, 128], F32, tag=f"selp{g}", name=f"selp{g}")
            nc.vector.scalar_tensor_tensor(selp[:], mp[:], -65536.0,
                                           cw[0:32, 137:265],
                                           op0=OP.mult, op1=OP.add)
            pstar = fpool.tile([32, 1], F32, tag=f"pstar{g}", name=f"pstar{g}")
            nc.vector.tensor_reduce(pstar[:], selp[:], axis=AX.X, op=OP.min)

            # flat row = pstar*64 + hm, wrapped to int16, 8x replicated
            hmc = 265 if g == 0 else 404
            rowf = fpool.tile([32, 1], F32, tag=f"rowf{g}", name=f"rowf{g}")
            nc.vector.scalar_tensor_tensor(rowf[:], pstar[:], 64.0,
                                           cw[0:32, hmc:hmc + 1],
                                           op0=OP.mult, op1=OP.add)
            R2 = fpool.tile([32, 2], F32, tag=f"R2{g}", name=f"R2{g}")
            nc.vector.tensor_scalar(R2[:], cw[0:32, 274:276], rowf[:], None,
                                    op0=OP.mult)
            IWp = mmps.tile([128, 2], F32, tag=f"IW{g}", name=f"IW{g}")
            nc.tensor.matmul(IWp[:], cw[0:32, 276:404], R2[:],
                             start=True, stop=True)
            idxw = fpool.tile([128, 2], I16, tag=f"idxw{g}", name=f"idxw{g}")
            nc.vector.tensor_copy(idxw[:], IWp[:])

            G = fpool.tile([128, C], F32, tag=f"G{g}", name=f"G{g}")
            nc.gpsimd.dma_gather(
                G[:].rearrange("p (o c) -> p o c", o=1),
                tgt_rows, idxw[:], num_idxs=32, num_idxs_reg=32, elem_size=C,
            )
            inmax8 = fpool.tile([32, 8], F32, tag=f"inmax8{g}",
                                name=f"inmax8{g}")
            nc.vector.tensor_scalar(inmax8[:], cw[0:32, 266:274], mh[:], None,
                                    op0=OP.mult)
            st.update(mh=mh, pstar=pstar, G=G, inmax8=inmax8)

        def res_b(g, out_eng):
            g0 = 32 * g
            st = half[g]
            pstar, G, inmax8 = st["pstar"], st["G"], st["inmax8"]
            ci8 = fpool.tile([32, 8], U16, tag=f"ci8{g}", name=f"ci8{g}")
            nc.vector.max_index(ci8[:], inmax8[:], G[0:32, :])
            cstar = fpool.tile([32, 1], F32, tag=f"cstar{g}", name=f"cstar{g}")
            nc.vector.tensor_copy(cstar[:], ci8[:, 0:1])

            bsel = fpool.tile([32, 1], F32, tag=f"bsel{g}", name=f"bsel{g}")
            nc.vector.tensor_scalar(bsel[:], cstar[:], 256.0, None,
                                    op0=OP.is_ge)
            wI = fpool.tile([32, 1], F32, tag=f"wI{g}", name=f"wI{g}")
            nc.vector.scalar_tensor_tensor(wI[:], bsel[:], -256.0, cstar[:],
                                           op0=OP.mult, op1=OP.add)
            hI = fpool.tile([32, 1], F32, tag=f"hI{g}", name=f"hI{g}")
            nc.vector.scalar_tensor_tensor(hI[:], pstar[:], 2.0, bsel[:],
                                           op0=OP.mult, op1=OP.add)
            tx = fpool.tile([32, 1], F32, tag=f"tx{g}", name=f"tx{g}")
            nc.vector.tensor_scalar(tx[:], wI[:], 2.0 / 256.0, -255.0 / 256.0,
                                    op0=OP.mult, op1=OP.add)
            ty = fpool.tile([32, 1], F32, tag=f"ty{g}", name=f"ty{g}")
            nc.vector.tensor_scalar(ty[:], hI[:], 2.0 / 256.0, -255.0 / 256.0,
                                    op0=OP.mult, op1=OP.add)

            rs = fpool.tile([32, 1], F32, tag=f"rs{g}", name=f"rs{g}")
            nc.vector.reciprocal(rs[:], S12[g][:, 0:1])
            px = fpool.tile([32, 1], F32, tag=f"px{g}", name=f"px{g}")
            nc.vector.tensor_mul(px[:], S12[g][:, 1:2], rs[:])
            py = fpool.tile([32, 1], F32, tag=f"py{g}", name=f"py{g}")
            nc.vector.tensor_mul(py[:], S12[g][:, 2:3], rs[:])
            dx = fpool.tile([32, 1], F32, tag=f"dx{g}", name=f"dx{g}")
            nc.vector.tensor_sub(dx[:], tx[:], px[:])
            dy = fpool.tile([32, 1], F32, tag=f"dy{g}", name=f"dy{g}")
            nc.vector.tensor_sub(dy[:], ty[:], py[:])
            dx2 = fpool.tile([32, 1], F32, tag=f"dx2{g}", name=f"dx2{g}")
            nc.vector.tensor_mul(dx2[:], dx[:], dx[:])
            dy2 = fpool.tile([32, 1], F32, tag=f"dy2{g}", name=f"dy2{g}")
            nc.vector.tensor_mul(dy2[:], dy[:], dy[:])
            nc.vector.tensor_add(ed2[g][:], dx2[:], dy2[:])
            out_eng.dma_start(out[g0:g0 + 32], ed2[g][:])

        # emit compute in issue order; half-0 resolution interleaves
        # mid-stream (after T3 completes RM[:,0:32]; piece B after T5)
        for pos, nm in enumerate(ORDER):
            k = int(nm[1:])
            if nm[0] == "I":
                input_compute(k)
            else:
                target_compute(k)
            if nm == "T3":
                res_a(0)
            elif nm == "T5":
                res_b(0, nc.gpsimd)
        res_a(1)
        res_b(1, nc.sync)

    nc.compile()
    return nc


_NC_CACHE = None


def _get_nc():
    global _NC_CACHE
    if _NC_CACHE is None:
        _NC_CACHE = build_nc()
    return _NC_CACHE


def make_in_maps(input, target):
    cw = make_consts()
    f8 = mybir.dt.np(F8)
    in_maps = []
    for i in range(NCORES):
        def shard(x, dt):
            # [4, 16, 256, 256] -> [128 part, 64*512] with col = hm*512 + c,
            # pixel (p, c): h = 2p + (c>=256), w = c%256  ->
            # heatmap[h, w] -> part p = h//2, col = (h%2)*256 + w
            s = x[i * BPC:(i + 1) * BPC].reshape(NHM, 128, 2, 256)
            s = s.transpose(1, 0, 2, 3).reshape(128, NHM * C)
            return np.ascontiguousarray(s.astype(dt))
        m = {"input": shard(input, f8),
             "target": shard(target, np.float32),
             "consts": cw}
        in_maps.append(m)
    return in_maps


def kernel(input, target, _trace=False):
    input = np.asarray(input, dtype=np.float32)
    target = np.asarray(target, dtype=np.float32)
    nc = _get_nc()
    in_maps = make_in_maps(input, target)
    r = run_bass_kernel_spmd(nc, in_maps, list(range(NCORES)), trace=_trace)
    total = np.float32(0.0)
    for res in r.results:
        ed = np.sqrt(res["out"].reshape(-1).astype(np.float32))
        total = np.float32(total + np.float32(ed.sum(dtype=np.float32)))
    out = np.array([total / np.float32(32.0)], dtype=np.float32)
    if _trace:
        return out, r
    return out

